# revision 3
# baseline (speedup 1.0000x reference)
"""Trainium2 Bass kernel for BasicQuadRGBModel (quad-Bayer demosaic CNN).

v4 design (bf16 matmuls, group-of-4-slab tiles, LDW-friendly ordering):
  - data parallel over 8 cores, 2 images each; per-image 16 groups of
    32 rows (4 slabs x 8 rows); pipeline stages A..D at group granularity.
  - im2col layout as before: conv = accumulating matmuls with K=120
    (10 xa x 12 ch band packing), M=96 (8 xo x 12 ch), N=512.
  - all matmul operands bf16 (PSUM accum fp32).
  - matmuls with the same stationary operand are emitted back-to-back
    (slab-inner loops) so weight reloads amortize.
  - softmax: 16-wide sum matmuls (wse16/wsep) + DVE reciprocal_approx_fast;
    E = max(exp(x), 1) replaces relu+exp.
  - chroma: one matmul per slab over a [104p] combined buffer (3 row-
    shifted copies of d = rb - g, plus the ky1 rows of r0) built by DMA.
  - f/w branch activations share one tile [120, 2, 34, 64] per layer so
    the xa-halo strip copies are 2 DMAs per layer per group.
  - r0 group tiles carry a +-1 row halo so the chroma d buffer is a
    single SBUF copy.
  - engine split: ACT = conv evicts + exp + chroma copy; DVE = P/E max,
    rcp, g mul, d sub; GpSimd = EP mul; DMA = strips/halos/combine/IO.
  - host does layer-0 im2col (bf16) and the final 2x2 pixel-shuffle.
"""

import sys

sys.path.insert(0, "/opt/trn_rl_repo")

import ml_dtypes
import numpy as np

import concourse.bass as bass  # noqa: F401
import concourse.mybir as mybir
import concourse.tile as tile
from concourse import bacc
from concourse.bass_utils import run_bass_kernel_spmd

N_CORES = 8
B_PC = 2
H = W = 512
NW = 64
CH = 12
GS = 4
GROWS = GS * 8
NG_IMG = H // GROWS
NGROUP = B_PC * NG_IMG
F32 = mybir.dt.float32
BF16 = mybir.dt.bfloat16
BF16NP = ml_dtypes.bfloat16


def _rbloc(xa, c):
    if xa == 0:
        return 16 + c
    if xa == 9:
        return 18 + c
    return (xa - 1) * 2 + c


def _rloc(xa, ci):
    if xa == 0:
        return 96 + ci
    if xa == 9:
        return 108 + ci
    return (xa - 1) * 12 + ci


def _r0loc(ky, ci, xa):
    if ky == 0:
        if ci == 0:
            return xa
        if ci == 3:
            return 10 + xa
        return 20 + _rbloc(xa, ci - 1)
    if ky == 1:
        if ci == 0:
            return 40 + xa
        if ci == 3:
            return 50 + xa
        return 64 + _rbloc(xa, ci - 1)
    if ci == 0:
        return 84 + xa
    if ci == 3:
        return 94 + xa
    return 104 + _rbloc(xa, ci - 1)


def build_r0(mosaic):
    B = mosaic.shape[0]
    mp = np.zeros((B, 4, H + 2, W + 2), BF16NP)
    mp[:, :, 1 : H + 1, 1 : W + 1] = mosaic.astype(BF16NP)
    r0 = np.zeros((B, 128, H, NW), BF16NP)
    for ky in range(3):
        for ci in range(4):
            for xa in range(10):
                r0[:, _r0loc(ky, ci, xa)] = mp[:, ci, ky : ky + H, xa : xa + 8 * NW : 8]
    return r0


def build_w_l0(wt):
    W_ = np.zeros((128, 96), np.float32)
    for ky in range(3):
        for ci in range(4):
            for xa in range(10):
                for xo in range(8):
                    kx = xa - xo
                    if 0 <= kx <= 2:
                        for co in range(CH):
                            W_[_r0loc(ky, ci, xa), xo * 12 + co] = wt[co, ci, ky, kx]
    return W_


def build_w_int(wt):
    W_ = np.zeros((3, 120, 96), np.float32)
    for ky in range(3):
        for xa in range(10):
            for xo in range(8):
                kx = xa - xo
                if 0 <= kx <= 2:
                    k = _rloc(xa, 0)
                    W_[ky, k : k + 12, xo * 12 : xo * 12 + 12] = wt[:, :, ky, kx].T
    return W_


def build_w_sums16():
    wse16 = np.zeros((96, 16), np.float32)
    wsep = np.zeros((96, 16), np.float32)
    for xo in range(8):
        for co in range(CH):
            wse16[xo * 12 + co, 2 * xo] = 1.0
            wse16[xo * 12 + co, 2 * xo + 1] = 1.0
            wsep[xo * 12 + co, xo * 2 + (co >= 6)] = 1.0
    return wse16, wsep


def build_w_chroma_comb(cw0):
    wchk = np.zeros((3, 20, 48), np.float32)
    for ky in range(3):
        for xa in range(10):
            for xo in range(8):
                kx = xa - xo
                if 0 <= kx <= 2:
                    for co in range(6):
                        for d in range(2):
                            wchk[ky, _rbloc(xa, d), xo * 6 + co] = cw0[co, d, ky, kx]
    # green_add = [m0, g1, m3, m0, g0, m3]; g0 = m1 - d0, g1 = m2 - d1
    for xo in range(8):
        wchk[1, _rbloc(xo + 1, 1), xo * 6 + 1] += -1.0
        wchk[1, _rbloc(xo + 1, 0), xo * 6 + 4] += -1.0
    wchm = np.zeros((128, 48), np.float32)
    for xo in range(8):
        xa = xo + 1
        wchm[_r0loc(1, 0, xa), xo * 6 + 0] = 1.0
        wchm[_r0loc(1, 0, xa), xo * 6 + 3] = 1.0
        wchm[_r0loc(1, 3, xa), xo * 6 + 2] = 1.0
        wchm[_r0loc(1, 3, xa), xo * 6 + 5] = 1.0
        wchm[_r0loc(1, 2, xa), xo * 6 + 1] = 1.0
        wchm[_r0loc(1, 1, xa), xo * 6 + 4] = 1.0
    Wc = np.zeros((104, 48), np.float32)
    for k in range(3):
        Wc[20 * k : 20 * k + 20] = wchk[k]
    Wc[60:104] = wchm[40:84]
    return Wc


def assemble_output(mosaic, cp_dev, g_dev):
    B = mosaic.shape[0]
    cp = (
        cp_dev.astype(np.float32)
        .reshape(B, 8, 6, H, NW)
        .transpose(0, 2, 3, 4, 1)
        .reshape(B, 6, H, W)
    )
    g = g_dev.reshape(B, 8, 2, H, NW).transpose(0, 2, 3, 4, 1).reshape(B, 2, H, W)
    m = mosaic
    out = np.empty((B, 3, 2 * H, 2 * W), np.float32)
    out[:, 0, 0::2, 0::2] = cp[:, 0]
    out[:, 0, 0::2, 1::2] = m[:, 1]
    out[:, 0, 1::2, 0::2] = cp[:, 1]
    out[:, 0, 1::2, 1::2] = cp[:, 2]
    out[:, 1, 0::2, 0::2] = m[:, 0]
    out[:, 1, 0::2, 1::2] = g[:, 0]
    out[:, 1, 1::2, 0::2] = g[:, 1]
    out[:, 1, 1::2, 1::2] = m[:, 3]
    out[:, 2, 0::2, 0::2] = cp[:, 3]
    out[:, 2, 0::2, 1::2] = cp[:, 4]
    out[:, 2, 1::2, 0::2] = m[:, 2]
    out[:, 2, 1::2, 1::2] = cp[:, 5]
    return out


# column offsets inside the packed [128, 1424] stationary tensor
_WOFF = {"wf0": 0, "ww0": 96, "wf1": 192, "wf2": 480, "ww1": 768, "ww2": 1056,
         "wse16": 1344, "wsep": 1360, "wcomb": 1376}
_WCOLS = 1424


def pack_stationaries(st):
    wp = np.zeros((128, _WCOLS), np.float32)
    wp[:, 0:96] = st["wf0"]
    wp[:, 96:192] = st["ww0"]
    for nm in ("wf1", "wf2", "ww1", "ww2"):
        o = _WOFF[nm]
        for ky in range(3):
            wp[0:120, o + 96 * ky : o + 96 * (ky + 1)] = st[nm][ky]
    wp[0:96, 1344:1360] = st["wse16"]
    wp[0:96, 1360:1376] = st["wsep"]
    wp[0:104, 1376:1424] = st["wcomb"]
    return wp


def build_program():
    from contextlib import ExitStack

    nc = bacc.Bacc(
        "TRN2", target_bir_lowering=False, debug=False, num_devices=N_CORES
    )
    r0 = nc.declare_dram_parameter("r0", [B_PC, 128, H, NW], BF16, isOutput=False)
    wpack = nc.declare_dram_parameter("wpack", [128, _WCOLS], BF16, isOutput=False)
    out_cp = nc.declare_dram_parameter("out_cp", [B_PC, 48, H, NW], BF16, isOutput=True)
    out_g = nc.declare_dram_parameter("out_g", [B_PC, 16, H, NW], F32, isOutput=True)

    Relu = mybir.ActivationFunctionType.Relu
    Exp = mybir.ActivationFunctionType.Exp
    Copy = mybir.ActivationFunctionType.Copy

    with tile.TileContext(nc) as tc, ExitStack() as ctx:
        const = ctx.enter_context(tc.tile_pool(name="const", bufs=1))
        r0pool = ctx.enter_context(tc.tile_pool(name="r0pool", bufs=5))
        p_r1 = ctx.enter_context(tc.tile_pool(name="r1", bufs=3))
        p_r2 = ctx.enter_context(tc.tile_pool(name="r2", bufs=3))
        p_grb = ctx.enter_context(tc.tile_pool(name="grb", bufs=3))
        p_d = ctx.enter_context(tc.tile_pool(name="dbuf", bufs=2))
        p_comb = ctx.enter_context(tc.tile_pool(name="comb", bufs=2))
        p_act = ctx.enter_context(tc.tile_pool(name="acts", bufs=4))
        p_rcp = ctx.enter_context(tc.tile_pool(name="rcp", bufs=4))
        p_stg = ctx.enter_context(tc.tile_pool(name="stg", bufs=2))
        ps_mm = ctx.enter_context(tc.tile_pool(name="psmm", bufs=4, space="PSUM"))
        ps_sm = ctx.enter_context(tc.tile_pool(name="pssm", bufs=2, space="PSUM"))
        ps_cp = ctx.enter_context(tc.tile_pool(name="pscp", bufs=2, space="PSUM"))

        WC = const.tile([128, _WCOLS], BF16, tag="wpack_sb", name="wpack_sb")
        nc.sync.dma_start(out=WC[:], in_=wpack[:])
        sb = {
            "wf0": WC[:, 0:96],
            "ww0": WC[:, 96:192],
            "wse16": WC[0:96, 1344:1360],
            "wsep": WC[0:96, 1360:1376],
            "wcomb": WC[0:104, 1376:1424],
        }

        def wky(nm, ky):
            o = _WOFF[nm]
            return WC[0:120, o + 96 * ky : o + 96 * (ky + 1)]

        r0s, r1, r2, grb = {}, {}, {}, {}

        def get_rbuf(pool, dct, g):
            if g in dct or not (0 <= g < NGROUP):
                return dct.get(g)
            t = pool.tile([120, 2, GROWS + 2, NW], BF16)
            dct[g] = t
            gi = g % NG_IMG
            if gi == 0:
                nc.vector.memset(t[0:96, :, 0:1, :], 0.0)
            if gi == NG_IMG - 1:
                nc.vector.memset(t[0:96, :, GROWS + 1 : GROWS + 2, :], 0.0)
            nc.vector.memset(t[96:120, :, :, 0:1], 0.0)
            nc.vector.memset(t[96:120, :, :, NW - 1 : NW], 0.0)
            return t

        def get_grb(g):
            if g in grb or not (0 <= g < NGROUP):
                return grb.get(g)
            t = p_grb.tile([20, GROWS + 2, NW], F32, name="g")
            grb[g] = t
            gi = g % NG_IMG
            if gi == 0:
                nc.vector.memset(t[:, 0:1, :], 0.0)
            if gi == NG_IMG - 1:
                nc.vector.memset(t[:, GROWS + 1 : GROWS + 2, :], 0.0)
            nc.vector.memset(t[:, :, 0:1], 0.0)
            nc.vector.memset(t[:, :, NW - 1 : NW], 0.0)
            return t

        def evict_g(ps, dct, br, g, gi, s):
            nc.scalar.activation(
                out=dct[g][0:96, br, 8 * s + 1 : 8 * s + 9, :], in_=ps[:], func=Relu
            )
            if s == 0 and gi > 0:
                nc.scalar.activation(
                    out=dct[g - 1][0:96, br, GROWS + 1 : GROWS + 2, :],
                    in_=ps[:, 0:1, :],
                    func=Relu,
                )
            if s == GS - 1 and gi < NG_IMG - 1:
                nc.scalar.activation(
                    out=dct[g + 1][0:96, br, 0:1, :], in_=ps[:, 7:8, :], func=Relu
                )

        def strips(t):
            nc.sync.dma_start(out=t[96:108, :, :, 1:NW], in_=t[84:96, :, :, 0 : NW - 1])
            nc.sync.dma_start(out=t[108:120, :, :, 0 : NW - 1], in_=t[0:12, :, :, 1:NW])

        for T in range(NGROUP + 3):
            a = T
            if 0 <= a < NGROUP:
                img, gi = divmod(a, NG_IMG)
                y0 = gi * GROWS
                rt = r0pool.tile([128, GROWS + 2, NW], BF16, name="rt")
                r0s[a] = rt
                if gi == 0:
                    nc.vector.memset(rt[:, 0:1, :], 0.0)
                    nc.sync.dma_start(
                        out=rt[:, 1 : GROWS + 2, :], in_=r0[img, :, 0 : GROWS + 1, :]
                    )
                elif gi == NG_IMG - 1:
                    nc.vector.memset(rt[:, GROWS + 1 : GROWS + 2, :], 0.0)
                    nc.sync.dma_start(
                        out=rt[:, 0 : GROWS + 1, :],
                        in_=r0[img, :, y0 - 1 : y0 + GROWS, :],
                    )
                else:
                    nc.sync.dma_start(
                        out=rt[:], in_=r0[img, :, y0 - 1 : y0 + GROWS + 1, :]
                    )
                get_rbuf(p_r1, r1, a)
                get_rbuf(p_r1, r1, a + 1)
                for br, nm in ((0, "wf0"), (1, "ww0")):
                    for s in range(GS):
                        ps = ps_mm.tile([96, 8, NW], F32, tag="mm96", name="ps0")
                        nc.tensor.matmul(
                            ps[:],
                            sb[nm],
                            rt[:, 8 * s + 1 : 8 * s + 9, :],
                            start=True,
                            stop=True,
                        )
                        evict_g(ps, r1, br, a, gi, s)

            b = T - 1
            if 0 <= b < NGROUP:
                img, gi = divmod(b, NG_IMG)
                strips(r1[b])
                get_rbuf(p_r2, r2, b)
                get_rbuf(p_r2, r2, b + 1)
                for br, nm in ((0, "wf1"), (1, "ww1")):
                    pss = [
                        ps_mm.tile([96, 8, NW], F32, tag="mm96", name="ps1")
                        for _ in range(GS)
                    ]
                    for ky in range(3):
                        wap = wky(nm, ky)
                        for s in range(GS):
                            nc.tensor.matmul(
                                pss[s][:],
                                wap,
                                r1[b][:, br, 8 * s + ky : 8 * s + ky + 8, :],
                                start=(ky == 0),
                                stop=(ky == 2),
                            )
                    for s in range(GS):
                        evict_g(pss[s], r2, br, b, gi, s)

            c = T - 2
            if 0 <= c < NGROUP:
                img, gi = divmod(c, NG_IMG)
                strips(r2[c])
                gt = get_grb(c)
                get_grb(c + 1)
                Ps, Es, EPs = [], [], []
                pss = [
                    ps_mm.tile([96, 8, NW], F32, tag="mm96", name="psf2")
                    for _ in range(GS)
                ]
                for ky in range(3):
                    wap = wky("wf2", ky)
                    for s in range(GS):
                        nc.tensor.matmul(
                            pss[s][:],
                            wap,
                            r2[c][:, 0, 8 * s + ky : 8 * s + ky + 8, :],
                            start=(ky == 0),
                            stop=(ky == 2),
                        )
                for s in range(GS):
                    P = p_act.tile([96, 8, NW], BF16, tag="P", name="P")
                    nc.vector.tensor_scalar_max(P[:], pss[s][:], 0.0)
                    Ps.append(P)
                pss = [
                    ps_mm.tile([96, 8, NW], F32, tag="mm96", name="psw2")
                    for _ in range(GS)
                ]
                for ky in range(3):
                    wap = wky("ww2", ky)
                    for s in range(GS):
                        nc.tensor.matmul(
                            pss[s][:],
                            wap,
                            r2[c][:, 1, 8 * s + ky : 8 * s + ky + 8, :],
                            start=(ky == 0),
                            stop=(ky == 2),
                        )
                for s in range(GS):
                    E0 = p_act.tile([96, 8, NW], BF16, tag="E0", name="E0")
                    nc.scalar.activation(out=E0[:], in_=pss[s][:], func=Exp)
                    E = p_act.tile([96, 8, NW], BF16, tag="E", name="E")
                    nc.vector.tensor_scalar_max(E[:], E0[:], 1.0)
                    Es.append(E)
                for s in range(GS):
                    EP = p_act.tile([96, 8, NW], BF16, tag="EP", name="EP")
                    nc.gpsimd.tensor_mul(EP[:], Es[s][:], Ps[s][:])
                    EPs.append(EP)
                rcps = []
                for s in range(GS):
                    pse = ps_sm.tile([16, 8, NW], F32, tag="sm", name="pse")
                    nc.tensor.matmul(
                        pse[:], sb["wse16"], Es[s][:], start=True, stop=True
                    )
                    rcp = p_rcp.tile([16, 8, NW], F32, tag="rcp", name="rcp")
                    nc.vector.reciprocal_approx_fast(out=rcp[:], in_=pse[:])
                    rcps.append(rcp)
                for s in range(GS):
                    psep = ps_sm.tile([16, 8, NW], F32, tag="sm", name="psep")
                    nc.tensor.matmul(
                        psep[:], sb["wsep"], EPs[s][:], start=True, stop=True
                    )
                    nc.vector.tensor_mul(
                        gt[0:16, 8 * s + 1 : 8 * s + 9, :], psep[:], rcps[s][:]
                    )
                    if s == 0 and gi > 0:
                        nc.vector.tensor_mul(
                            grb[c - 1][0:16, GROWS + 1 : GROWS + 2, :],
                            psep[:, 0:1, :],
                            rcps[s][:, 0:1, :],
                        )
                    if s == GS - 1 and gi < NG_IMG - 1:
                        nc.vector.tensor_mul(
                            grb[c + 1][0:16, 0:1, :],
                            psep[:, 7:8, :],
                            rcps[s][:, 7:8, :],
                        )

            g = T - 3
            if 0 <= g < NGROUP:
                img, gi = divmod(g, NG_IMG)
                gt = grb[g]
                nc.sync.dma_start(out=gt[16:18, :, 1:NW], in_=gt[14:16, :, 0 : NW - 1])
                nc.sync.dma_start(out=gt[18:20, :, 0 : NW - 1], in_=gt[0:2, :, 1:NW])
                dt = p_d.tile([20, GROWS + 2, NW], BF16, name="d")
                nc.sync.dma_start(out=dt[:], in_=r0s[g][64:84, :, :])
                nc.vector.tensor_sub(dt[:], dt[:], gt[:])
                cb = p_comb.tile([104, GROWS, NW], BF16, name="cb")
                for k in range(3):
                    nc.sync.dma_start(
                        out=cb[20 * k : 20 * (k + 1), :, :], in_=dt[:, k : k + GROWS, :]
                    )
                nc.sync.dma_start(
                    out=cb[60:104, :, :], in_=r0s[g][40:84, 1 : GROWS + 1, :]
                )
                stgt = p_stg.tile([48, GROWS, NW], BF16, name="stg")
                for s in range(GS):
                    pc = ps_cp.tile([48, 8, NW], F32, tag="cp", name="pc")
                    nc.tensor.matmul(
                        pc[:],
                        sb["wcomb"],
                        cb[:, 8 * s : 8 * s + 8, :],
                        start=True,
                        stop=True,
                    )
                    nc.scalar.activation(
                        out=stgt[:, 8 * s : 8 * s + 8, :], in_=pc[:], func=Copy
                    )
                y0 = gi * GROWS
                nc.sync.dma_start(out=out_cp[img, :, y0 : y0 + GROWS, :], in_=stgt[:])
                nc.sync.dma_start(
                    out=out_g[img, :, y0 : y0 + GROWS, :],
                    in_=gt[0:16, 1 : GROWS + 1, :],
                )
                r0s.pop(g, None)
                r1.pop(T - 2, None)
                r2.pop(T - 3, None)
                grb.pop(g, None)

    nc.compile()
    return nc


_CACHE = {}


def kernel(mosaic, fw0, fw1, fw2, ww0, ww1, ww2, cw0, _trace=False):
    mosaic = np.asarray(mosaic, np.float32)
    r0_all = build_r0(mosaic)

    stat = {
        "wf0": build_w_l0(np.asarray(fw0, np.float32)),
        "ww0": build_w_l0(np.asarray(ww0, np.float32)),
        "wf1": build_w_int(np.asarray(fw1, np.float32)),
        "wf2": build_w_int(np.asarray(fw2, np.float32)),
        "ww1": build_w_int(np.asarray(ww1, np.float32)),
        "ww2": build_w_int(np.asarray(ww2, np.float32)),
    }
    stat["wse16"], stat["wsep"] = build_w_sums16()
    stat["wcomb"] = build_w_chroma_comb(np.asarray(cw0, np.float32))
    wpack = pack_stationaries(stat).astype(BF16NP)

    if "nc" not in _CACHE:
        _CACHE["nc"] = build_program()
    nc = _CACHE["nc"]

    in_maps = []
    for c in range(N_CORES):
        in_maps.append(
            {"r0": np.ascontiguousarray(r0_all[c * B_PC : (c + 1) * B_PC]),
             "wpack": wpack}
        )

    res = run_bass_kernel_spmd(nc, in_maps, list(range(N_CORES)), trace=_trace)
    outs = []
    for c in range(N_CORES):
        outs.append(
            assemble_output(
                mosaic[c * B_PC : (c + 1) * B_PC],
                res.results[c]["out_cp"],
                res.results[c]["out_g"],
            )
        )
    full = np.concatenate(outs, axis=0)
    if _trace:
        return full, res
    return full
